# revision 33
# baseline (speedup 1.0000x reference)
"""Multi-head self-attention (RoPE, causal) Trainium2 Bass kernel.

Problem: B=4, S=2048, D=1024, H=16 heads, hd=64, fused QKV + RoPE +
causal softmax attention + output projection (torch-Linear convention).

Sharding: Megatron-style tensor parallel over heads. Each of the 8
NeuronCores owns 2 heads: it projects the full token stream through its
128-row slices of Wq/Wk/Wv, applies RoPE, runs causal attention for its
2 heads x 4 batches, and computes a partial output projection
h_core @ Wo[:, core_slice].T  (transposed layout). The host sums the 8
partial outputs and adds the output bias.

Per-core pipeline (restructured for PE density / HAM warmth):
  1. QKV projection per 512-token chunk: q,k produced in [d, tok] layout
     (weights stationary, wide moving streams), RoPE via a pair-rotation
     matmul + DVE combines; v in natural [tok, d] layout (x-tiles
     stationary) with a ones column appended for softmax sums.
  2. Scores transposed sT[k,q] = kT.T@qT per (batch, head): the two
     heads' K=64 matmuls are emitted pairwise so they run concurrently
     in the two row-halves of the PE array (tile_position row tiling).
     exp on ACT (scale fused); causal diagonal blocks masked on GpSimd.
  3. P@V V-stationary: out[65, q] = sum_kt vA[kt].T @ P_T[kt-row], with
     per-512-q-window PSUM accumulators; row 64 carries the softmax
     denominators. Drain: copy rows 0-63 to SBUF, reciprocal of row 64,
     GpSimd partition_broadcast of the reciprocal row, DVE multiply
     producing normalized hT[d, tok] (head 1 shifted to partitions
     64-127 by a small SBUF-to-SBUF DMA).
  4. Output projection, transposed: partial.T[f, tok] = WoT.T @ hT,
     PSUM -> SBUF casts alternating DVE/ACT, DMA to DRAM.
Emission interleaves phases so the PE queue never drains (HAM 2.4GHz).
"""

import os
import sys

for _p in ("/opt/trn_rl_repo",):
    if os.path.isdir(_p) and _p not in sys.path:
        sys.path.append(_p)

import math

import ml_dtypes
import numpy as np

import concourse.bass as bass
import concourse.mybir as mybir
import concourse.tile as tile
from concourse import bacc
from concourse.bass import ts, ds
from concourse.bass_utils import run_bass_kernel_spmd

BF16 = ml_dtypes.bfloat16

B = 4
S = 2048
D = 1024
H = 16
HD = 64
NCORES = 8
HPC = H // NCORES          # heads per core = 2
PC = HPC * HD              # partition rows per core's heads = 128
T = B * S                  # 8192 tokens
KT = D // 128              # f_in k-tiles = 8
NTOK = T // 128            # 64 token tiles of 128
SCALE = 1.0 / math.sqrt(HD)
ROPE_THETA = 10000.0

TWO_PI = 2.0 * math.pi
INV_2PI = 1.0 / TWO_PI
MAGIC = 12582912.0         # 1.5 * 2**23, float32 round-to-nearest trick
HALF_PI = math.pi / 2.0

NQT = S // 128             # 16 k/q tiles per sequence
WW = 512                   # PV q-window width
NW = S // WW               # 4 windows per (batch, head)
# triangular packing offsets for P_T: row kt covers q in [kt*128, S)
OFFS = [0] * NQT
for _kt in range(1, NQT):
    OFFS[_kt] = OFFS[_kt - 1] + (S - (_kt - 1) * 128)
PTRI_W = OFFS[-1] + (S - (NQT - 1) * 128)   # 17408

TC = 512                   # projection token chunk
NTC = T // TC              # 16 chunks
CPB = S // TC              # chunks per batch = 4


def _row_chunks(kt):
    """512-grid-aligned q chunks covering [128*kt, S)."""
    out = []
    lo = 128 * kt
    while lo < S:
        hi = min(S, (lo // 512 + 1) * 512)
        out.append((lo, hi))
        lo = hi
    return out


def _build_nc(dbg=False):
    nc = bacc.Bacc("TRN2", target_bir_lowering=False, debug=False,
                   num_devices=NCORES)
    dt = mybir.dt

    # ---- I/O ----
    x_in = nc.dram_tensor("x", [D, T], dt.bfloat16, kind="ExternalInput")
    pos_in = nc.dram_tensor("pos", [S], dt.int32, kind="ExternalInput")
    wq_in = nc.dram_tensor("wq", [D, PC], dt.bfloat16, kind="ExternalInput")
    wk_in = nc.dram_tensor("wk", [D, PC], dt.bfloat16, kind="ExternalInput")
    wv_in = nc.dram_tensor("wv", [D, PC], dt.bfloat16, kind="ExternalInput")
    wo_in = nc.dram_tensor("wo", [PC, D], dt.bfloat16, kind="ExternalInput")
    bq_in = nc.dram_tensor("bq", [PC], dt.float32, kind="ExternalInput")
    bk_in = nc.dram_tensor("bk", [PC], dt.float32, kind="ExternalInput")
    bv_in = nc.dram_tensor("bv", [PC], dt.float32, kind="ExternalInput")
    out_d = nc.dram_tensor("out", [D, T], dt.bfloat16, kind="ExternalOutput")
    # DRAM scratch for the per-window softmax-reciprocal rows (bounce buffer
    # for the partition broadcast; SBUF stride-0 DMA is not supported)
    rs_d = nc.dram_tensor("rscratch", [B * HPC * NW, WW], dt.float32,
                          kind="Internal")
    if dbg:
        dbg_qT = nc.dram_tensor("dbg_qT", [128, T], dt.bfloat16,
                                kind="ExternalOutput")
        dbg_kT = nc.dram_tensor("dbg_kT", [128, T], dt.bfloat16,
                                kind="ExternalOutput")
        dbg_vA = nc.dram_tensor("dbg_vA", [128, NTOK * HPC * (HD + 1)],
                                dt.bfloat16, kind="ExternalOutput")
        dbg_pt = nc.dram_tensor("dbg_pt", [128, HPC * PTRI_W], dt.bfloat16,
                                kind="ExternalOutput")
        dbg_hT = nc.dram_tensor("dbg_hT", [128, T], dt.bfloat16,
                                kind="ExternalOutput")

    # ---- inline constants ----
    # RT = R.T where (R @ q)[2i] = -q[2i+1], (R @ q)[2i+1] = q[2i],
    # block-diagonal over the 2 stacked heads.
    r = np.zeros((PC, PC), dtype=np.float32)
    for h in range(HPC):
        for i in range(HD // 2):
            r[h * HD + 2 * i, h * HD + 2 * i + 1] = -1.0
            r[h * HD + 2 * i + 1, h * HD + 2 * i] = 1.0
    rt_np = np.ascontiguousarray(r.T).astype(BF16)
    # causal mask for diagonal scoresT blocks: keep k_local <= q_local
    mask_np = np.tril(np.ones((128, 128), dtype=np.float32)).T.astype(BF16)
    # inv_freq per partition: p -> head-local pair (p % 64) // 2
    invf_np = np.zeros((PC, 1), dtype=np.float32)
    for p in range(PC):
        i = (p % HD) // 2
        invf_np[p, 0] = 1.0 / (ROPE_THETA ** (2.0 * i / HD))

    rt_d = nc.inline_tensor(rt_np, "rt_c")
    mask_d = nc.inline_tensor(mask_np, "mask_c")
    invf_d = nc.inline_tensor(invf_np, "invf_c")

    fp32 = dt.float32
    bf16 = dt.bfloat16

    with tile.TileContext(nc) as tc:
        with (
            tc.tile_pool(name="consts", bufs=1) as consts,
            tc.tile_pool(name="resid", bufs=1) as resid,
            tc.tile_pool(name="xp", bufs=2) as xp,
            tc.tile_pool(name="wk3", bufs=3) as wk3,
            tc.tile_pool(name="pvw", bufs=2) as pvw,
            tc.tile_pool(name="ow", bufs=3) as ow,
            tc.tile_pool(name="csw", bufs=1) as csw,
            tc.tile_pool(name="ptri", bufs=1) as ptri_pool,
            tc.tile_pool(name="scps", bufs=2, space="PSUM") as scps,
            tc.tile_pool(name="pvps", bufs=2, space="PSUM") as pvps,
            tc.tile_pool(name="pjps", bufs=2, space="PSUM") as pjps,
            tc.tile_pool(name="vps", bufs=2, space="PSUM") as vps,
        ):
            # ---- load constants / weights to SBUF ----
            wq_sb = consts.tile([128, KT, PC], bf16, tag="wq")
            wk_sb = consts.tile([128, KT, PC], bf16, tag="wk")
            wv_sb = consts.tile([128, KT, PC], bf16, tag="wv")
            for t_sb, t_d in ((wq_sb, wq_in), (wk_sb, wk_in), (wv_sb, wv_in)):
                nc.sync.dma_start(
                    out=t_sb, in_=t_d.ap().rearrange("(kt p) m -> p kt m", p=128))
            wo_sb = consts.tile([128, D], bf16, tag="wo")
            nc.sync.dma_start(out=wo_sb, in_=wo_in[:, :])
            rt_sb = consts.tile([128, 128], bf16, tag="rt")
            nc.sync.dma_start(out=rt_sb, in_=rt_d[:, :])
            mask_sb = consts.tile([128, 128], bf16, tag="mask")
            nc.sync.dma_start(out=mask_sb, in_=mask_d[:, :])
            invf_sb = consts.tile([128, 1], fp32, tag="invf")
            nc.sync.dma_start(out=invf_sb, in_=invf_d[:, :])
            bq_sb = consts.tile([128, 1], fp32, tag="bq")
            nc.sync.dma_start(out=bq_sb, in_=bq_in.ap().rearrange("(p o) -> p o", o=1))
            bk_sb = consts.tile([128, 1], fp32, tag="bk")
            nc.sync.dma_start(out=bk_sb, in_=bk_in.ap().rearrange("(p o) -> p o", o=1))
            # v bias broadcast over token partitions: [128, PC] f32
            bvb_sb = consts.tile([128, PC], fp32, tag="bvb")
            nc.sync.dma_start(
                out=bvb_sb,
                in_=bass.AP(tensor=bv_in, offset=0, ap=[[0, 128], [1, PC]]))
            halfpi_sb = consts.tile([128, 1], fp32, tag="halfpi")
            nc.vector.memset(halfpi_sb, HALF_PI)

            # ---- RoPE cos/sin tables [128, S] bf16, from positions ----
            cos_sb = consts.tile([128, S], bf16, tag="cosT")
            sin_sb = consts.tile([128, S], bf16, tag="sinT")
            CS_CH = 512
            for ci in range(S // CS_CH):
                sl = ts(ci, CS_CH)
                posi = csw.tile([128, CS_CH], dt.int32, tag="posi")
                nc.sync.dma_start(
                    out=posi,
                    in_=bass.AP(tensor=pos_in, offset=ci * CS_CH,
                                ap=[[0, 128], [1, CS_CH]]))
                posf = csw.tile([128, CS_CH], fp32, tag="posf")
                nc.vector.tensor_copy(posf, posi)
                ang = csw.tile([128, CS_CH], fp32, tag="ang")
                nc.vector.tensor_scalar_mul(ang, posf, invf_sb)
                # sin: reduce ang to [-pi, pi]
                rnd = csw.tile([128, CS_CH], fp32, tag="rnd")
                red = csw.tile([128, CS_CH], fp32, tag="red")
                nc.vector.tensor_scalar(rnd, ang, INV_2PI, MAGIC,
                                        mybir.AluOpType.mult,
                                        mybir.AluOpType.add)
                nc.vector.tensor_scalar(rnd, rnd, MAGIC, None,
                                        mybir.AluOpType.subtract)
                nc.vector.scalar_tensor_tensor(
                    red, rnd, -TWO_PI, ang,
                    op0=mybir.AluOpType.mult, op1=mybir.AluOpType.add)
                nc.scalar.activation(sin_sb[:, sl], red,
                                     mybir.ActivationFunctionType.Sin)
                # cos(x) = sin(y + pi/2), y = x - 2pi*round((x+pi/2)/2pi)
                nc.vector.tensor_scalar(rnd, ang, INV_2PI, MAGIC + 0.25,
                                        mybir.AluOpType.mult,
                                        mybir.AluOpType.add)
                nc.vector.tensor_scalar(rnd, rnd, MAGIC, None,
                                        mybir.AluOpType.subtract)
                nc.vector.scalar_tensor_tensor(
                    red, rnd, -TWO_PI, ang,
                    op0=mybir.AluOpType.mult, op1=mybir.AluOpType.add)
                nc.scalar.activation(cos_sb[:, sl], red,
                                     mybir.ActivationFunctionType.Sin,
                                     bias=halfpi_sb[:, :])

            # ---- residents ----
            qT = resid.tile([128, T], bf16, tag="qT")     # [d(2h), tok]
            kT = resid.tile([128, T], bf16, tag="kT")
            hT = resid.tile([128, T], bf16, tag="hT")
            # v natural + ones column: [tok%128, tok//128, head, 65]
            vA = resid.tile([128, NTOK, HPC, HD + 1], bf16, tag="vA")
            nc.vector.memset(vA[:, :, :, HD:HD + 1], 1.0)

            xTr = x_in.ap().rearrange("(kt p) n -> p kt n", p=128)

            # ---- phase 1 (itemized): QKV projection + RoPE for one
            # 512-token chunk; returns a list of emission closures so the
            # scheduler can weave them between score chunks. ----
            def phase1_items(tci):
                tsl = ts(tci, TC)
                ssl = ds((tci * TC) % S, TC)
                xt = xp.tile([128, KT, TC], bf16, tag="xt")
                items = [lambda: nc.sync.dma_start(out=xt, in_=xTr[:, :, tsl])]

                def qk(w_sb, b_sb, dest):
                    # pa is reused for the rotation product pb (the rope
                    # matmul overwrites it after the bias-add drains it)
                    sub = []
                    pa = pjps.tile([128, TC], fp32, tag="pj")
                    for kt in range(KT):
                        sub.append(lambda kt=kt, pa=pa: nc.tensor.matmul(
                            pa, lhsT=w_sb[:, kt, :], rhs=xt[:, kt, :],
                            start=(kt == 0), stop=(kt == KT - 1)))
                    a_sb = wk3.tile([128, TC], bf16, tag="a_sb")

                    def fin1(pa=pa, a_sb=a_sb):
                        nc.vector.tensor_scalar_add(a_sb, pa, b_sb)
                        nc.tensor.matmul(pa, lhsT=rt_sb, rhs=a_sb,
                                         start=True, stop=True)
                    sub.append(fin1)

                    def fin2(a_sb=a_sb, pa=pa, dest=dest):
                        t1 = wk3.tile([128, TC], bf16, tag="t1")
                        nc.vector.tensor_mul(t1, a_sb, cos_sb[:, ssl])
                        t2 = wk3.tile([128, TC], bf16, tag="t2")
                        nc.vector.tensor_mul(t2, pa, sin_sb[:, ssl])
                        nc.vector.tensor_add(dest[:, tsl], t1, t2)
                    sub.append(fin2)
                    return sub

                items += qk(wq_sb, bq_sb, qT)
                items += qk(wk_sb, bk_sb, kT)

                # v: natural layout; one full-bank psum tile per token tile
                # (start=True clears the whole bank, so regions of one bank
                # cannot host independent accumulation chains)
                for sub in range(TC // 128):
                    pv = vps.tile([128, TC], fp32, tag="v",
                                  name=f"pv_{tci}_{sub}")

                    def vstep(sub=sub, pv=pv):
                        for kt in range(KT):
                            nc.tensor.matmul(
                                pv[:, 0:PC],
                                lhsT=xt[:, kt, ds(sub * 128, 128)],
                                rhs=wv_sb[:, kt, :],
                                start=(kt == 0), stop=(kt == KT - 1))
                    items.append(vstep)

                    def vdrain(pv=pv, tt=tci * (TC // 128) + sub):
                        nc.vector.tensor_add(
                            vA[:, tt, :, 0:HD],
                            pv[:, 0:PC].rearrange("p (h d) -> p h d", h=HPC),
                            bvb_sb.rearrange("p (h d) -> p h d", h=HPC))
                    items.append(vdrain)
                return items

            # ---- phase 2 slot: one (kt, q-chunk) for both heads. The two
            # K=64 matmuls land on disjoint PE row-halves (auto row
            # tiling) so they execute concurrently; one dual-head exp
            # drains both PSUM banks in a single ACT instruction. ----
            def p2_slot(b, kt, lo, hi, ptc):
                base = b * S
                w = hi - lo
                off = OFFS[kt] + lo - 128 * kt
                scs = []
                for h in range(HPC):
                    hsl = ds(h * HD, HD)
                    sc = scps.tile([128, 512], fp32, tag="sc",
                                   name=f"sc_{b}_{kt}_{lo}_{h}")
                    nc.tensor.matmul(
                        sc[:, 0:w],
                        lhsT=kT[hsl, ds(base + kt * 128, 128)],
                        rhs=qT[hsl, ds(base + lo, w)],
                        start=True, stop=True)
                    scs.append(sc)
                for h in range(HPC):
                    nc.scalar.activation(
                        ptc[:, h, ds(off, w)], scs[h][:, 0:w],
                        mybir.ActivationFunctionType.Exp, scale=SCALE)
                if lo == 128 * kt:
                    # diagonal block: mask k_local > q_local on GpSimd
                    for h in range(HPC):
                        dsl = ds(OFFS[kt], 128)
                        nc.gpsimd.tensor_mul(ptc[:, h, dsl], ptc[:, h, dsl],
                                             mask_sb)

            # ---- phase 3: one PV q-window for one (b, h) ----
            def phase3(b, h, w, ptc):
                pt = ptc[:, h, :]
                kmax = (w * WW + WW - 1) // 128   # last kt contributing
                pvt = pvps.tile([128, WW], fp32, tag="pv")
                for kt in range(kmax + 1):
                    qlo = max(128 * kt, w * WW)
                    width = w * WW + WW - qlo
                    nc.tensor.matmul(
                        pvt[0:HD + 1, ds(qlo - w * WW, width)],
                        lhsT=vA[:, b * NQT + kt, h, :],
                        rhs=pt[:, ds(OFFS[kt] + qlo - 128 * kt, width)],
                        start=(kt == 0), stop=(kt == kmax))
                # drain: copy rows 0-64 (incl. sums row) to SBUF, wide
                # approx reciprocal, then broadcast the reciprocal row to 64
                # partitions via a DRAM bounce (write row, read stride-0).
                stg = pvw.tile([128, WW], fp32, tag="stg")
                nc.vector.tensor_copy(stg[0:HD + 1, :], pvt[0:HD + 1, :])
                rcp = pvw.tile([128, WW], fp32, tag="rcp")
                nc.vector.reciprocal_approx_fast(out=rcp, in_=stg)
                ri = (b * HPC + h) * NW + w
                nc.gpsimd.dma_start(out=rs_d[ri, :], in_=rcp[HD:HD + 1, :])
                rbc = pvw.tile([HD, WW], fp32, tag="rbc")
                nc.gpsimd.dma_start(
                    out=rbc,
                    in_=bass.AP(tensor=rs_d, offset=ri * WW,
                                ap=[[0, HD], [1, WW]]))
                span = ds(b * S + w * WW, WW)
                if h == 0:
                    nc.vector.tensor_mul(hT[0:HD, span], stg[0:HD, :], rbc)
                else:
                    hst = pvw.tile([HD, WW], bf16, tag="hst")
                    nc.vector.tensor_mul(hst, stg[0:HD, :], rbc)
                    nc.gpsimd.dma_start(out=hT[HD:128, span], in_=hst)

            # ---- phase 4: output projection for one batch ----
            # cc-outer so chunks over early PV windows don't wait on the
            # last window's drain
            def phase4(b):
                base = b * S
                n = 0
                for cc in range(S // 512):
                    for ft in range(D // 128):
                        po = pjps.tile([128, 512], fp32, tag="pj")
                        nc.tensor.matmul(
                            po, lhsT=wo_sb[:, ts(ft, 128)],
                            rhs=hT[:, ds(base + cc * 512, 512)],
                            start=True, stop=True)
                        ostage = ow.tile([128, 512], bf16, tag="ostage")
                        if n % 2 == 0:
                            nc.vector.tensor_copy(ostage, po)
                        else:
                            nc.scalar.copy(ostage, po)
                        nc.sync.dma_start(
                            out=out_d[ts(ft, 128), ds(base + cc * 512, 512)],
                            in_=ostage)
                        n += 1

            # ---- master schedule ----
            # prologue: batch 0 projection
            for it in phase1_items(0):
                it()
            for it in phase1_items(1):
                it()
            for it in phase1_items(2):
                it()
            for it in phase1_items(3):
                it()

            from collections import deque

            for b in range(B):
                # next batch's projection items, woven between score slots
                # so the PE always has queued work while ACT drains exps
                fillers = deque()
                if b + 1 < B:
                    for cc in range(CPB):
                        fillers.extend(phase1_items((b + 1) * CPB + cc))
                ptc = ptri_pool.tile([128, HPC, PTRI_W], bf16, tag="pt",
                                     name=f"pt_b{b}")
                slots_left = sum(len(_row_chunks(kt)) for kt in range(NQT))
                for g in range(NW):           # 4 row-groups of 4 kt rows
                    for kt in range(4 * g, 4 * g + 4):
                        for (lo, hi) in _row_chunks(kt):
                            p2_slot(b, kt, lo, hi, ptc)
                            if fillers and slots_left > 0:
                                want = -(-len(fillers) // slots_left)
                                for _ in range(want):
                                    if fillers:
                                        fillers.popleft()()
                            slots_left -= 1
                    # this row-group complete: its PV window is ready
                    for h in range(HPC):
                        phase3(b, h, g, ptc)
                while fillers:
                    fillers.popleft()()
                if dbg and b == 0:
                    nc.sync.dma_start(out=dbg_pt[:, :],
                                      in_=ptc.rearrange("p h w -> p (h w)"))
                phase4(b)
            if dbg:
                nc.sync.dma_start(out=dbg_qT[:, :], in_=qT)
                nc.sync.dma_start(out=dbg_kT[:, :], in_=kT)
                nc.sync.dma_start(out=dbg_vA[:, :],
                                  in_=vA.rearrange("p a h d -> p (a h d)"))
                nc.sync.dma_start(out=dbg_hT[:, :], in_=hT)

    nc.compile()
    return nc


_NC_CACHE = None


def _get_nc():
    global _NC_CACHE
    if _NC_CACHE is None:
        _NC_CACHE = _build_nc()
    return _NC_CACHE


def kernel(x, positions, Wqkv, bqkv, Wo, bo):
    x = np.asarray(x)
    positions = np.asarray(positions)
    Wqkv = np.asarray(Wqkv)
    bqkv = np.asarray(bqkv)
    Wo = np.asarray(Wo)
    bo = np.asarray(bo)

    nc = _get_nc()

    xT = np.ascontiguousarray(x.reshape(T, D).T).astype(BF16)
    pos = np.ascontiguousarray(positions[0]).astype(np.int32)

    in_maps = []
    for c in range(NCORES):
        r0 = c * PC
        wq = np.ascontiguousarray(Wqkv[r0:r0 + PC, :].T).astype(BF16)
        wk = np.ascontiguousarray(Wqkv[D + r0:D + r0 + PC, :].T).astype(BF16)
        wv = np.ascontiguousarray(Wqkv[2 * D + r0:2 * D + r0 + PC, :].T).astype(BF16)
        wo = np.ascontiguousarray(Wo[:, r0:r0 + PC].T).astype(BF16)
        in_maps.append({
            "x": xT, "pos": pos,
            "wq": wq, "wk": wk, "wv": wv, "wo": wo,
            "bq": bqkv[r0:r0 + PC].astype(np.float32),
            "bk": bqkv[D + r0:D + r0 + PC].astype(np.float32),
            "bv": bqkv[2 * D + r0:2 * D + r0 + PC].astype(np.float32),
        })

    res = run_bass_kernel_spmd(nc, in_maps, core_ids=list(range(NCORES)))
    acc = res.results[0]["out"].astype(np.float32)
    for c in range(1, NCORES):
        acc += res.results[c]["out"].astype(np.float32)
    out = acc + bo[:, None].astype(np.float32)
    return np.ascontiguousarray(out.T).reshape(B, S, D)


# revision 37
# speedup vs baseline: 1.0440x; 1.0440x over previous
"""Multi-head self-attention (RoPE, causal) Trainium2 Bass kernel.

Problem: B=4, S=2048, D=1024, H=16 heads, hd=64, fused QKV + RoPE +
causal softmax attention + output projection (torch-Linear convention).

Sharding: Megatron-style tensor parallel over heads. Each of the 8
NeuronCores owns 2 heads: it projects the full token stream through its
128-row slices of Wq/Wk/Wv, applies RoPE, runs causal attention for its
2 heads x 4 batches, and computes a partial output projection
h_core @ Wo[:, core_slice].T  (transposed layout). The host sums the 8
partial outputs and adds the output bias.

Per-core pipeline (restructured for PE density / HAM warmth):
  1. QKV projection per 512-token chunk: q,k produced in [d, tok] layout
     (weights stationary, wide moving streams), RoPE via a pair-rotation
     matmul + DVE combines; v in natural [tok, d] layout (x-tiles
     stationary) with a ones column appended for softmax sums.
  2. Scores transposed sT[k,q] = kT.T@qT per (batch, head): the two
     heads' K=64 matmuls are emitted pairwise so they run concurrently
     in the two row-halves of the PE array (tile_position row tiling).
     exp on ACT (scale fused); causal diagonal blocks masked on GpSimd.
  3. P@V V-stationary: out[65, q] = sum_kt vA[kt].T @ P_T[kt-row], with
     per-512-q-window PSUM accumulators; row 64 carries the softmax
     denominators. Drain: copy rows 0-63 to SBUF, reciprocal of row 64,
     GpSimd partition_broadcast of the reciprocal row, DVE multiply
     producing normalized hT[d, tok] (head 1 shifted to partitions
     64-127 by a small SBUF-to-SBUF DMA).
  4. Output projection, transposed: partial.T[f, tok] = WoT.T @ hT,
     PSUM -> SBUF casts alternating DVE/ACT, DMA to DRAM.
Emission interleaves phases so the PE queue never drains (HAM 2.4GHz).
"""

import os
import sys

for _p in ("/opt/trn_rl_repo",):
    if os.path.isdir(_p) and _p not in sys.path:
        sys.path.append(_p)

import math

import ml_dtypes
import numpy as np

import concourse.bass as bass
import concourse.mybir as mybir
import concourse.tile as tile
from concourse import bacc
from concourse.bass import ts, ds
from concourse.bass_utils import run_bass_kernel_spmd

BF16 = ml_dtypes.bfloat16

B = 4
S = 2048
D = 1024
H = 16
HD = 64
NCORES = 8
HPC = H // NCORES          # heads per core = 2
PC = HPC * HD              # partition rows per core's heads = 128
T = B * S                  # 8192 tokens
KT = D // 128              # f_in k-tiles = 8
NTOK = T // 128            # 64 token tiles of 128
SCALE = 1.0 / math.sqrt(HD)
ROPE_THETA = 10000.0

TWO_PI = 2.0 * math.pi
INV_2PI = 1.0 / TWO_PI
MAGIC = 12582912.0         # 1.5 * 2**23, float32 round-to-nearest trick
HALF_PI = math.pi / 2.0

NQT = S // 128             # 16 k/q tiles per sequence
WW = 512                   # PV q-window width
NW = S // WW               # 4 windows per (batch, head)
# triangular packing offsets for P_T: row kt covers q in [kt*128, S)
OFFS = [0] * NQT
for _kt in range(1, NQT):
    OFFS[_kt] = OFFS[_kt - 1] + (S - (_kt - 1) * 128)
PTRI_W = OFFS[-1] + (S - (NQT - 1) * 128)   # 17408

TC = 512                   # projection token chunk
NTC = T // TC              # 16 chunks
CPB = S // TC              # chunks per batch = 4


def _row_chunks(kt):
    """512-grid-aligned q chunks covering [128*kt, S)."""
    out = []
    lo = 128 * kt
    while lo < S:
        hi = min(S, (lo // 512 + 1) * 512)
        out.append((lo, hi))
        lo = hi
    return out


def _build_nc(dbg=False):
    nc = bacc.Bacc("TRN2", target_bir_lowering=False, debug=False,
                   num_devices=NCORES)
    dt = mybir.dt

    # ---- I/O ----
    x_in = nc.dram_tensor("x", [D, T], dt.bfloat16, kind="ExternalInput")
    pos_in = nc.dram_tensor("pos", [S], dt.int32, kind="ExternalInput")
    wq_in = nc.dram_tensor("wq", [D, PC], dt.bfloat16, kind="ExternalInput")
    wk_in = nc.dram_tensor("wk", [D, PC], dt.bfloat16, kind="ExternalInput")
    wv_in = nc.dram_tensor("wv", [D, PC], dt.bfloat16, kind="ExternalInput")
    wo_in = nc.dram_tensor("wo", [PC, D], dt.bfloat16, kind="ExternalInput")
    bq_in = nc.dram_tensor("bq", [PC], dt.float32, kind="ExternalInput")
    bk_in = nc.dram_tensor("bk", [PC], dt.float32, kind="ExternalInput")
    bv_in = nc.dram_tensor("bv", [PC], dt.float32, kind="ExternalInput")
    out_d = nc.dram_tensor("out", [D, T], dt.bfloat16, kind="ExternalOutput")
    # DRAM scratch for the per-window softmax-reciprocal rows (bounce buffer
    # for the partition broadcast; SBUF stride-0 DMA is not supported)
    rs_d = nc.dram_tensor("rscratch", [B * HPC * NW, WW], dt.float32,
                          kind="Internal")
    if dbg:
        dbg_qT = nc.dram_tensor("dbg_qT", [128, T], dt.bfloat16,
                                kind="ExternalOutput")
        dbg_kT = nc.dram_tensor("dbg_kT", [128, T], dt.bfloat16,
                                kind="ExternalOutput")
        dbg_vA = nc.dram_tensor("dbg_vA", [128, NTOK * HPC * (HD + 1)],
                                dt.bfloat16, kind="ExternalOutput")
        dbg_pt = nc.dram_tensor("dbg_pt", [128, HPC * PTRI_W], dt.bfloat16,
                                kind="ExternalOutput")
        dbg_hT = nc.dram_tensor("dbg_hT", [128, T], dt.bfloat16,
                                kind="ExternalOutput")

    # ---- inline constants ----
    # RT = R.T where (R @ q)[2i] = -q[2i+1], (R @ q)[2i+1] = q[2i],
    # block-diagonal over the 2 stacked heads.
    r = np.zeros((PC, PC), dtype=np.float32)
    for h in range(HPC):
        for i in range(HD // 2):
            r[h * HD + 2 * i, h * HD + 2 * i + 1] = -1.0
            r[h * HD + 2 * i + 1, h * HD + 2 * i] = 1.0
    rt_np = np.ascontiguousarray(r.T).astype(BF16)
    # causal mask for diagonal scoresT blocks: keep k_local <= q_local
    mask_np = np.tril(np.ones((128, 128), dtype=np.float32)).T.astype(BF16)
    # inv_freq per partition: p -> head-local pair (p % 64) // 2
    invf_np = np.zeros((PC, 1), dtype=np.float32)
    for p in range(PC):
        i = (p % HD) // 2
        invf_np[p, 0] = 1.0 / (ROPE_THETA ** (2.0 * i / HD))

    rt_d = nc.inline_tensor(rt_np, "rt_c")
    mask_d = nc.inline_tensor(mask_np, "mask_c")
    invf_d = nc.inline_tensor(invf_np, "invf_c")

    fp32 = dt.float32
    bf16 = dt.bfloat16

    with tile.TileContext(nc) as tc:
        with (
            tc.tile_pool(name="consts", bufs=1) as consts,
            tc.tile_pool(name="resid", bufs=1) as resid,
            tc.tile_pool(name="xp", bufs=2) as xp,
            tc.tile_pool(name="wk3", bufs=3) as wk3,
            tc.tile_pool(name="pvw", bufs=2) as pvw,
            tc.tile_pool(name="ow", bufs=3) as ow,
            tc.tile_pool(name="csw", bufs=1) as csw,
            tc.tile_pool(name="ptri", bufs=1) as ptri_pool,
            tc.tile_pool(name="scps", bufs=2, space="PSUM") as scps,
            tc.tile_pool(name="pvps", bufs=2, space="PSUM") as pvps,
            tc.tile_pool(name="pjps", bufs=2, space="PSUM") as pjps,
            tc.tile_pool(name="vps", bufs=2, space="PSUM") as vps,
        ):
            # ---- load constants / weights to SBUF ----
            wq_sb = consts.tile([128, KT, PC], bf16, tag="wq")
            wk_sb = consts.tile([128, KT, PC], bf16, tag="wk")
            wv_sb = consts.tile([128, KT, PC], bf16, tag="wv")
            for t_sb, t_d in ((wq_sb, wq_in), (wk_sb, wk_in), (wv_sb, wv_in)):
                nc.sync.dma_start(
                    out=t_sb, in_=t_d.ap().rearrange("(kt p) m -> p kt m", p=128))
            wo_sb = consts.tile([128, D], bf16, tag="wo")
            nc.sync.dma_start(out=wo_sb, in_=wo_in[:, :])
            rt_sb = consts.tile([128, 128], bf16, tag="rt")
            nc.sync.dma_start(out=rt_sb, in_=rt_d[:, :])
            mask_sb = consts.tile([128, 128], bf16, tag="mask")
            nc.sync.dma_start(out=mask_sb, in_=mask_d[:, :])
            invf_sb = consts.tile([128, 1], fp32, tag="invf")
            nc.sync.dma_start(out=invf_sb, in_=invf_d[:, :])
            bq_sb = consts.tile([128, 1], fp32, tag="bq")
            nc.sync.dma_start(out=bq_sb, in_=bq_in.ap().rearrange("(p o) -> p o", o=1))
            bk_sb = consts.tile([128, 1], fp32, tag="bk")
            nc.sync.dma_start(out=bk_sb, in_=bk_in.ap().rearrange("(p o) -> p o", o=1))
            # v bias broadcast over token partitions: [128, PC] f32
            bvb_sb = consts.tile([128, PC], fp32, tag="bvb")
            nc.sync.dma_start(
                out=bvb_sb,
                in_=bass.AP(tensor=bv_in, offset=0, ap=[[0, 128], [1, PC]]))
            halfpi_sb = consts.tile([128, 1], fp32, tag="halfpi")
            nc.vector.memset(halfpi_sb, HALF_PI)

            # ---- RoPE cos/sin tables [128, S] bf16, from positions ----
            cos_sb = consts.tile([128, S], bf16, tag="cosT")
            sin_sb = consts.tile([128, S], bf16, tag="sinT")
            CS_CH = 512
            for ci in range(S // CS_CH):
                sl = ts(ci, CS_CH)
                posi = csw.tile([128, CS_CH], dt.int32, tag="posi")
                nc.sync.dma_start(
                    out=posi,
                    in_=bass.AP(tensor=pos_in, offset=ci * CS_CH,
                                ap=[[0, 128], [1, CS_CH]]))
                posf = csw.tile([128, CS_CH], fp32, tag="posf")
                nc.vector.tensor_copy(posf, posi)
                ang = csw.tile([128, CS_CH], fp32, tag="ang")
                nc.vector.tensor_scalar_mul(ang, posf, invf_sb)
                # sin: reduce ang to [-pi, pi]
                rnd = csw.tile([128, CS_CH], fp32, tag="rnd")
                red = csw.tile([128, CS_CH], fp32, tag="red")
                nc.vector.tensor_scalar(rnd, ang, INV_2PI, MAGIC,
                                        mybir.AluOpType.mult,
                                        mybir.AluOpType.add)
                nc.vector.tensor_scalar(rnd, rnd, MAGIC, None,
                                        mybir.AluOpType.subtract)
                nc.vector.scalar_tensor_tensor(
                    red, rnd, -TWO_PI, ang,
                    op0=mybir.AluOpType.mult, op1=mybir.AluOpType.add)
                nc.scalar.activation(sin_sb[:, sl], red,
                                     mybir.ActivationFunctionType.Sin)
                # cos(x) = sin(y + pi/2), y = x - 2pi*round((x+pi/2)/2pi)
                nc.vector.tensor_scalar(rnd, ang, INV_2PI, MAGIC + 0.25,
                                        mybir.AluOpType.mult,
                                        mybir.AluOpType.add)
                nc.vector.tensor_scalar(rnd, rnd, MAGIC, None,
                                        mybir.AluOpType.subtract)
                nc.vector.scalar_tensor_tensor(
                    red, rnd, -TWO_PI, ang,
                    op0=mybir.AluOpType.mult, op1=mybir.AluOpType.add)
                nc.scalar.activation(cos_sb[:, sl], red,
                                     mybir.ActivationFunctionType.Sin,
                                     bias=halfpi_sb[:, :])

            # ---- residents ----
            qT = resid.tile([128, T], bf16, tag="qT")     # [d(2h), tok]
            kT = resid.tile([128, T], bf16, tag="kT")
            hT = resid.tile([128, T], bf16, tag="hT")
            # v natural + ones column: [tok%128, tok//128, head, 65]
            vA = resid.tile([128, NTOK, HPC, HD + 1], bf16, tag="vA")
            nc.vector.memset(vA[:, :, :, HD:HD + 1], 1.0)

            xTr = x_in.ap().rearrange("(kt p) n -> p kt n", p=128)

            # ---- phase 1 (itemized): QKV projection + RoPE for one
            # 512-token chunk; returns a list of emission closures so the
            # scheduler can weave them between score chunks. ----
            def phase1_items(tci):
                tsl = ts(tci, TC)
                ssl = ds((tci * TC) % S, TC)
                xt = xp.tile([128, KT, TC], bf16, tag="xt")
                items = [lambda: nc.sync.dma_start(out=xt, in_=xTr[:, :, tsl])]

                def qk(w_sb, b_sb, dest):
                    # pa is reused for the rotation product pb (the rope
                    # matmul overwrites it after the bias-add drains it)
                    sub = []
                    pa = pjps.tile([128, TC], fp32, tag="pj")
                    for kt in range(KT):
                        sub.append(lambda kt=kt, pa=pa: nc.tensor.matmul(
                            pa, lhsT=w_sb[:, kt, :], rhs=xt[:, kt, :],
                            start=(kt == 0), stop=(kt == KT - 1)))
                    a_sb = wk3.tile([128, TC], bf16, tag="a_sb")

                    def fin1(pa=pa, a_sb=a_sb):
                        nc.vector.tensor_scalar_add(a_sb, pa, b_sb)
                        nc.tensor.matmul(pa, lhsT=rt_sb, rhs=a_sb,
                                         start=True, stop=True)
                    sub.append(fin1)

                    def fin2(a_sb=a_sb, pa=pa, dest=dest):
                        t1 = wk3.tile([128, TC], bf16, tag="t1")
                        nc.vector.tensor_mul(t1, a_sb, cos_sb[:, ssl])
                        t2 = wk3.tile([128, TC], bf16, tag="t2")
                        nc.vector.tensor_mul(t2, pa, sin_sb[:, ssl])
                        nc.vector.tensor_add(dest[:, tsl], t1, t2)
                    sub.append(fin2)
                    return sub

                items += qk(wq_sb, bq_sb, qT)
                items += qk(wk_sb, bk_sb, kT)

                # v: natural layout; one full-bank psum tile per token tile
                # (start=True clears the whole bank, so regions of one bank
                # cannot host independent accumulation chains)
                for sub in range(TC // 128):
                    pv = vps.tile([128, TC], fp32, tag="v",
                                  name=f"pv_{tci}_{sub}")

                    def vstep(sub=sub, pv=pv):
                        for kt in range(KT):
                            nc.tensor.matmul(
                                pv[:, 0:PC],
                                lhsT=xt[:, kt, ds(sub * 128, 128)],
                                rhs=wv_sb[:, kt, :],
                                start=(kt == 0), stop=(kt == KT - 1))
                    items.append(vstep)

                    def vdrain(pv=pv, tt=tci * (TC // 128) + sub):
                        nc.vector.tensor_add(
                            vA[:, tt, :, 0:HD],
                            pv[:, 0:PC].rearrange("p (h d) -> p h d", h=HPC),
                            bvb_sb.rearrange("p (h d) -> p h d", h=HPC))
                    items.append(vdrain)
                return items

            # ---- phase 2 slot: one (kt, q-chunk) for both heads. The two
            # K=64 matmuls land on disjoint PE row-halves (auto row
            # tiling) so they execute concurrently. ----
            def p2_slot(b, kt, lo, hi, pts):
                base = b * S
                w = hi - lo
                off = OFFS[kt] + lo - 128 * kt
                scs = []
                for h in range(HPC):
                    hsl = ds(h * HD, HD)
                    sc = scps.tile([128, 512], fp32, tag="sc",
                                   name=f"sc_{b}_{kt}_{lo}_{h}")
                    nc.tensor.matmul(
                        sc[:, 0:w],
                        lhsT=kT[hsl, ds(base + kt * 128, 128)],
                        rhs=qT[hsl, ds(base + lo, w)],
                        start=True, stop=True)
                    scs.append(sc)
                for h in range(HPC):
                    nc.scalar.activation(
                        pts[h][:, ds(off, w)], scs[h][:, 0:w],
                        mybir.ActivationFunctionType.Exp, scale=SCALE)
                if lo == 128 * kt:
                    # diagonal block: mask k_local > q_local on GpSimd
                    for h in range(HPC):
                        dsl = ds(OFFS[kt], 128)
                        nc.gpsimd.tensor_mul(pts[h][:, dsl], pts[h][:, dsl],
                                             mask_sb)

            # ---- phase 3: one PV q-window for one (b, h) ----
            def phase3(b, h, w, pt):
                kmax = (w * WW + WW - 1) // 128   # last kt contributing
                pvt = pvps.tile([128, WW], fp32, tag="pv")
                for kt in range(kmax + 1):
                    qlo = max(128 * kt, w * WW)
                    width = w * WW + WW - qlo
                    nc.tensor.matmul(
                        pvt[0:HD + 1, ds(qlo - w * WW, width)],
                        lhsT=vA[:, b * NQT + kt, h, :],
                        rhs=pt[:, ds(OFFS[kt] + qlo - 128 * kt, width)],
                        start=(kt == 0), stop=(kt == kmax))
                # drain: copy rows 0-64 (incl. sums row) to SBUF, wide
                # approx reciprocal, then broadcast the reciprocal row to 64
                # partitions via a DRAM bounce (write row, read stride-0).
                stg = pvw.tile([128, WW], fp32, tag="stg")
                nc.vector.tensor_copy(stg[0:HD + 1, :], pvt[0:HD + 1, :])
                rcp = pvw.tile([128, WW], fp32, tag="rcp")
                nc.vector.reciprocal_approx_fast(out=rcp, in_=stg)
                ri = (b * HPC + h) * NW + w
                nc.gpsimd.dma_start(out=rs_d[ri, :], in_=rcp[HD:HD + 1, :])
                rbc = pvw.tile([HD, WW], fp32, tag="rbc")
                nc.gpsimd.dma_start(
                    out=rbc,
                    in_=bass.AP(tensor=rs_d, offset=ri * WW,
                                ap=[[0, HD], [1, WW]]))
                span = ds(b * S + w * WW, WW)
                if h == 0:
                    nc.vector.tensor_mul(hT[0:HD, span], stg[0:HD, :], rbc)
                else:
                    hst = pvw.tile([HD, WW], bf16, tag="hst")
                    nc.vector.tensor_mul(hst, stg[0:HD, :], rbc)
                    nc.gpsimd.dma_start(out=hT[HD:128, span], in_=hst)

            # ---- phase 4 (itemized): output projection chunks for one
            # batch, cc-outer; woven into the next phase as PE filler ----
            def phase4_items(b):
                base = b * S
                items = []
                n = 0
                for cc in range(S // 512):
                    for ft in range(D // 128):
                        def p4(ft=ft, cc=cc, n=n):
                            po = pjps.tile([128, 512], fp32, tag="pj",
                                           name=f"po_{b}_{cc}_{ft}")
                            nc.tensor.matmul(
                                po, lhsT=wo_sb[:, ts(ft, 128)],
                                rhs=hT[:, ds(base + cc * 512, 512)],
                                start=True, stop=True)
                            ostage = ow.tile([128, 512], bf16, tag="ostage",
                                             name=f"os_{b}_{cc}_{ft}")
                            if n % 2 == 0:
                                nc.vector.tensor_copy(ostage, po)
                            else:
                                nc.scalar.copy(ostage, po)
                            nc.sync.dma_start(
                                out=out_d[ts(ft, 128),
                                          ds(base + cc * 512, 512)],
                                in_=ostage)
                        items.append(p4)
                        n += 1
                return items

            # ---- master schedule ----
            # prologue: batch 0 projection
            for it in phase1_items(0):
                it()
            for it in phase1_items(1):
                it()
            for it in phase1_items(2):
                it()
            for it in phase1_items(3):
                it()

            from collections import deque

            p4_carry = []
            for b in range(B):
                # weave the next batch's projection and the previous
                # batch's output projection between score slots so the PE
                # always has queued work while ACT drains exps
                fillers = deque()
                p1f = deque()
                if b + 1 < B:
                    for cc in range(CPB):
                        p1f.extend(phase1_items((b + 1) * CPB + cc))
                p4f = deque(p4_carry)
                # round-robin merge, ~2 p1 items per p4 item
                while p1f or p4f:
                    for _ in range(2):
                        if p1f:
                            fillers.append(p1f.popleft())
                    if p4f:
                        fillers.append(p4f.popleft())
                pts = [ptri_pool.tile([128, PTRI_W], bf16, tag=f"pt{h}",
                                      name=f"pt_b{b}h{h}")
                       for h in range(HPC)]
                slots_left = sum(len(_row_chunks(kt)) for kt in range(NQT))
                for g in range(NW):           # 4 row-groups of 4 kt rows
                    for kt in range(4 * g, 4 * g + 4):
                        for (lo, hi) in _row_chunks(kt):
                            p2_slot(b, kt, lo, hi, pts)
                            if fillers and slots_left > 0:
                                want = -(-len(fillers) // slots_left)
                                for _ in range(want):
                                    if fillers:
                                        fillers.popleft()()
                            slots_left -= 1
                    # this row-group complete: its PV window is ready
                    for h in range(HPC):
                        phase3(b, h, g, pts[h])
                while fillers:
                    fillers.popleft()()
                if dbg and b == 0:
                    nc.sync.dma_start(out=dbg_pt[:, 0:PTRI_W], in_=pts[0])
                    nc.sync.dma_start(out=dbg_pt[:, PTRI_W:2 * PTRI_W],
                                      in_=pts[1])
                p4_carry = phase4_items(b)
            for it in p4_carry:
                it()
            if dbg:
                nc.sync.dma_start(out=dbg_qT[:, :], in_=qT)
                nc.sync.dma_start(out=dbg_kT[:, :], in_=kT)
                nc.sync.dma_start(out=dbg_vA[:, :],
                                  in_=vA.rearrange("p a h d -> p (a h d)"))
                nc.sync.dma_start(out=dbg_hT[:, :], in_=hT)

    nc.compile()
    return nc


_NC_CACHE = None


def _get_nc():
    global _NC_CACHE
    if _NC_CACHE is None:
        _NC_CACHE = _build_nc()
    return _NC_CACHE


def kernel(x, positions, Wqkv, bqkv, Wo, bo):
    x = np.asarray(x)
    positions = np.asarray(positions)
    Wqkv = np.asarray(Wqkv)
    bqkv = np.asarray(bqkv)
    Wo = np.asarray(Wo)
    bo = np.asarray(bo)

    nc = _get_nc()

    xT = np.ascontiguousarray(x.reshape(T, D).T).astype(BF16)
    pos = np.ascontiguousarray(positions[0]).astype(np.int32)

    in_maps = []
    for c in range(NCORES):
        r0 = c * PC
        wq = np.ascontiguousarray(Wqkv[r0:r0 + PC, :].T).astype(BF16)
        wk = np.ascontiguousarray(Wqkv[D + r0:D + r0 + PC, :].T).astype(BF16)
        wv = np.ascontiguousarray(Wqkv[2 * D + r0:2 * D + r0 + PC, :].T).astype(BF16)
        wo = np.ascontiguousarray(Wo[:, r0:r0 + PC].T).astype(BF16)
        in_maps.append({
            "x": xT, "pos": pos,
            "wq": wq, "wk": wk, "wv": wv, "wo": wo,
            "bq": bqkv[r0:r0 + PC].astype(np.float32),
            "bk": bqkv[D + r0:D + r0 + PC].astype(np.float32),
            "bv": bqkv[2 * D + r0:2 * D + r0 + PC].astype(np.float32),
        })

    res = run_bass_kernel_spmd(nc, in_maps, core_ids=list(range(NCORES)))
    acc = res.results[0]["out"].astype(np.float32)
    for c in range(1, NCORES):
        acc += res.results[c]["out"].astype(np.float32)
    out = acc + bo[:, None].astype(np.float32)
    return np.ascontiguousarray(out.T).reshape(B, S, D)


# revision 41
# speedup vs baseline: 1.0648x; 1.0199x over previous
"""Multi-head self-attention (RoPE, causal) Trainium2 Bass kernel.

Problem: B=4, S=2048, D=1024, H=16 heads, hd=64, fused QKV + RoPE +
causal softmax attention + output projection (torch-Linear convention).

Sharding: Megatron-style tensor parallel over heads. Each of the 8
NeuronCores owns 2 heads: it projects the full token stream through its
128-row slices of Wq/Wk/Wv, applies RoPE, runs causal attention for its
2 heads x 4 batches, and computes a partial output projection
h_core @ Wo[:, core_slice].T  (transposed layout). The host sums the 8
partial outputs and adds the output bias.

Per-core pipeline (restructured for PE density / HAM warmth):
  1. QKV projection per 512-token chunk: q,k produced in [d, tok] layout
     (weights stationary, wide moving streams), RoPE via a pair-rotation
     matmul + DVE combines; v in natural [tok, d] layout (x-tiles
     stationary) with a ones column appended for softmax sums.
  2. Scores transposed sT[k,q] = kT.T@qT per (batch, head): the two
     heads' K=64 matmuls are emitted pairwise so they run concurrently
     in the two row-halves of the PE array (tile_position row tiling).
     exp on ACT (scale fused); causal diagonal blocks masked on GpSimd.
  3. P@V V-stationary: out[65, q] = sum_kt vA[kt].T @ P_T[kt-row], with
     per-512-q-window PSUM accumulators; row 64 carries the softmax
     denominators. Drain: copy rows 0-63 to SBUF, reciprocal of row 64,
     GpSimd partition_broadcast of the reciprocal row, DVE multiply
     producing normalized hT[d, tok] (head 1 shifted to partitions
     64-127 by a small SBUF-to-SBUF DMA).
  4. Output projection, transposed: partial.T[f, tok] = WoT.T @ hT,
     PSUM -> SBUF casts alternating DVE/ACT, DMA to DRAM.
Emission interleaves phases so the PE queue never drains (HAM 2.4GHz).
"""

import os
import sys

for _p in ("/opt/trn_rl_repo",):
    if os.path.isdir(_p) and _p not in sys.path:
        sys.path.append(_p)

import math

import ml_dtypes
import numpy as np

import concourse.bass as bass
import concourse.mybir as mybir
import concourse.tile as tile
from concourse import bacc
from concourse.bass import ts, ds
from concourse.bass_utils import run_bass_kernel_spmd

BF16 = ml_dtypes.bfloat16

B = 4
S = 2048
D = 1024
H = 16
HD = 64
NCORES = 8
HPC = H // NCORES          # heads per core = 2
PC = HPC * HD              # partition rows per core's heads = 128
T = B * S                  # 8192 tokens
KT = D // 128              # f_in k-tiles = 8
NTOK = T // 128            # 64 token tiles of 128
SCALE = 1.0 / math.sqrt(HD)
ROPE_THETA = 10000.0

TWO_PI = 2.0 * math.pi
INV_2PI = 1.0 / TWO_PI
MAGIC = 12582912.0         # 1.5 * 2**23, float32 round-to-nearest trick
HALF_PI = math.pi / 2.0

NQT = S // 128             # 16 k/q tiles per sequence
WW = 512                   # PV q-window width
NW = S // WW               # 4 windows per (batch, head)
# triangular packing offsets for P_T: row kt covers q in [kt*128, S)
OFFS = [0] * NQT
for _kt in range(1, NQT):
    OFFS[_kt] = OFFS[_kt - 1] + (S - (_kt - 1) * 128)
PTRI_W = OFFS[-1] + (S - (NQT - 1) * 128)   # 17408

TC = 512                   # projection token chunk
NTC = T // TC              # 16 chunks
CPB = S // TC              # chunks per batch = 4


def _row_chunks(kt):
    """512-grid-aligned q chunks covering [128*kt, S)."""
    out = []
    lo = 128 * kt
    while lo < S:
        hi = min(S, (lo // 512 + 1) * 512)
        out.append((lo, hi))
        lo = hi
    return out


def _build_nc(dbg=False):
    nc = bacc.Bacc("TRN2", target_bir_lowering=False, debug=False,
                   num_devices=NCORES)
    dt = mybir.dt

    # ---- I/O ----
    x_in = nc.dram_tensor("x", [D, T], dt.bfloat16, kind="ExternalInput")
    pos_in = nc.dram_tensor("pos", [S], dt.int32, kind="ExternalInput")
    wq_in = nc.dram_tensor("wq", [D, PC], dt.bfloat16, kind="ExternalInput")
    wk_in = nc.dram_tensor("wk", [D, PC], dt.bfloat16, kind="ExternalInput")
    wv_in = nc.dram_tensor("wv", [D, PC], dt.bfloat16, kind="ExternalInput")
    wo_in = nc.dram_tensor("wo", [PC, D], dt.bfloat16, kind="ExternalInput")
    bq_in = nc.dram_tensor("bq", [PC], dt.float32, kind="ExternalInput")
    bk_in = nc.dram_tensor("bk", [PC], dt.float32, kind="ExternalInput")
    bv_in = nc.dram_tensor("bv", [PC], dt.float32, kind="ExternalInput")
    out_d = nc.dram_tensor("out", [D, T], dt.bfloat16, kind="ExternalOutput")
    # DRAM scratch for the per-window softmax-reciprocal rows (bounce buffer
    # for the partition broadcast; SBUF stride-0 DMA is not supported)
    rs_d = nc.dram_tensor("rscratch", [B * HPC * NW, WW], dt.float32,
                          kind="Internal")
    if dbg:
        dbg_qT = nc.dram_tensor("dbg_qT", [128, T], dt.bfloat16,
                                kind="ExternalOutput")
        dbg_kT = nc.dram_tensor("dbg_kT", [128, T], dt.bfloat16,
                                kind="ExternalOutput")
        dbg_vA = nc.dram_tensor("dbg_vA", [128, NTOK * HPC * (HD + 1)],
                                dt.bfloat16, kind="ExternalOutput")
        dbg_pt = nc.dram_tensor("dbg_pt", [128, HPC * PTRI_W], dt.bfloat16,
                                kind="ExternalOutput")
        dbg_hT = nc.dram_tensor("dbg_hT", [128, T], dt.bfloat16,
                                kind="ExternalOutput")

    # ---- inline constants ----
    # RT = R.T where (R @ q)[2i] = -q[2i+1], (R @ q)[2i+1] = q[2i],
    # block-diagonal over the 2 stacked heads.
    r = np.zeros((PC, PC), dtype=np.float32)
    for h in range(HPC):
        for i in range(HD // 2):
            r[h * HD + 2 * i, h * HD + 2 * i + 1] = -1.0
            r[h * HD + 2 * i + 1, h * HD + 2 * i] = 1.0
    rt_np = np.ascontiguousarray(r.T).astype(BF16)
    # causal mask for diagonal scoresT blocks: keep k_local <= q_local
    mask_np = np.tril(np.ones((128, 128), dtype=np.float32)).T.astype(BF16)
    # inv_freq per partition: p -> head-local pair (p % 64) // 2
    invf_np = np.zeros((PC, 1), dtype=np.float32)
    for p in range(PC):
        i = (p % HD) // 2
        invf_np[p, 0] = 1.0 / (ROPE_THETA ** (2.0 * i / HD))

    rt_d = nc.inline_tensor(rt_np, "rt_c")
    mask_d = nc.inline_tensor(mask_np, "mask_c")
    invf_d = nc.inline_tensor(invf_np, "invf_c")

    fp32 = dt.float32
    bf16 = dt.bfloat16

    with tile.TileContext(nc) as tc:
        with (
            tc.tile_pool(name="consts", bufs=1) as consts,
            tc.tile_pool(name="resid", bufs=1) as resid,
            tc.tile_pool(name="xp", bufs=2) as xp,
            tc.tile_pool(name="wk3", bufs=3) as wk3,
            tc.tile_pool(name="pvw", bufs=2) as pvw,
            tc.tile_pool(name="ow", bufs=3) as ow,
            tc.tile_pool(name="csw", bufs=1) as csw,
            tc.tile_pool(name="ptri", bufs=1) as ptri_pool,
            tc.tile_pool(name="scps", bufs=1, space="PSUM") as scps,
            tc.tile_pool(name="pvps", bufs=2, space="PSUM") as pvps,
            tc.tile_pool(name="pjps", bufs=2, space="PSUM") as pjps,
            tc.tile_pool(name="vps", bufs=2, space="PSUM") as vps,
        ):
            # ---- load constants / weights to SBUF ----
            wq_sb = consts.tile([128, KT, PC], bf16, tag="wq")
            wk_sb = consts.tile([128, KT, PC], bf16, tag="wk")
            wv_sb = consts.tile([128, KT, PC], bf16, tag="wv")
            for t_sb, t_d in ((wq_sb, wq_in), (wk_sb, wk_in), (wv_sb, wv_in)):
                nc.sync.dma_start(
                    out=t_sb, in_=t_d.ap().rearrange("(kt p) m -> p kt m", p=128))
            wo_sb = consts.tile([128, D], bf16, tag="wo")
            nc.sync.dma_start(out=wo_sb, in_=wo_in[:, :])
            rt_sb = consts.tile([128, 128], bf16, tag="rt")
            nc.sync.dma_start(out=rt_sb, in_=rt_d[:, :])
            mask_sb = consts.tile([128, 128], bf16, tag="mask")
            nc.sync.dma_start(out=mask_sb, in_=mask_d[:, :])
            invf_sb = consts.tile([128, 1], fp32, tag="invf")
            nc.sync.dma_start(out=invf_sb, in_=invf_d[:, :])
            bq_sb = consts.tile([128, 1], fp32, tag="bq")
            nc.sync.dma_start(out=bq_sb, in_=bq_in.ap().rearrange("(p o) -> p o", o=1))
            bk_sb = consts.tile([128, 1], fp32, tag="bk")
            nc.sync.dma_start(out=bk_sb, in_=bk_in.ap().rearrange("(p o) -> p o", o=1))
            # v bias broadcast over token partitions: [128, PC] f32
            bvb_sb = consts.tile([128, PC], fp32, tag="bvb")
            nc.sync.dma_start(
                out=bvb_sb,
                in_=bass.AP(tensor=bv_in, offset=0, ap=[[0, 128], [1, PC]]))
            halfpi_sb = consts.tile([128, 1], fp32, tag="halfpi")
            nc.vector.memset(halfpi_sb, HALF_PI)

            # ---- RoPE cos/sin tables [128, S] bf16, from positions ----
            cos_sb = consts.tile([128, S], bf16, tag="cosT")
            sin_sb = consts.tile([128, S], bf16, tag="sinT")
            CS_CH = 512
            for ci in range(S // CS_CH):
                sl = ts(ci, CS_CH)
                posi = csw.tile([128, CS_CH], dt.int32, tag="posi")
                nc.sync.dma_start(
                    out=posi,
                    in_=bass.AP(tensor=pos_in, offset=ci * CS_CH,
                                ap=[[0, 128], [1, CS_CH]]))
                posf = csw.tile([128, CS_CH], fp32, tag="posf")
                nc.vector.tensor_copy(posf, posi)
                ang = csw.tile([128, CS_CH], fp32, tag="ang")
                nc.vector.tensor_scalar_mul(ang, posf, invf_sb)
                # sin: reduce ang to [-pi, pi]
                rnd = csw.tile([128, CS_CH], fp32, tag="rnd")
                red = csw.tile([128, CS_CH], fp32, tag="red")
                nc.vector.tensor_scalar(rnd, ang, INV_2PI, MAGIC,
                                        mybir.AluOpType.mult,
                                        mybir.AluOpType.add)
                nc.vector.tensor_scalar(rnd, rnd, MAGIC, None,
                                        mybir.AluOpType.subtract)
                nc.vector.scalar_tensor_tensor(
                    red, rnd, -TWO_PI, ang,
                    op0=mybir.AluOpType.mult, op1=mybir.AluOpType.add)
                nc.scalar.activation(sin_sb[:, sl], red,
                                     mybir.ActivationFunctionType.Sin)
                # cos(x) = sin(y + pi/2), y = x - 2pi*round((x+pi/2)/2pi)
                nc.vector.tensor_scalar(rnd, ang, INV_2PI, MAGIC + 0.25,
                                        mybir.AluOpType.mult,
                                        mybir.AluOpType.add)
                nc.vector.tensor_scalar(rnd, rnd, MAGIC, None,
                                        mybir.AluOpType.subtract)
                nc.vector.scalar_tensor_tensor(
                    red, rnd, -TWO_PI, ang,
                    op0=mybir.AluOpType.mult, op1=mybir.AluOpType.add)
                nc.scalar.activation(cos_sb[:, sl], red,
                                     mybir.ActivationFunctionType.Sin,
                                     bias=halfpi_sb[:, :])

            # ---- residents ----
            qT = resid.tile([128, T], bf16, tag="qT")     # [d(2h), tok]
            kT = resid.tile([128, T], bf16, tag="kT")
            hT = resid.tile([128, T], bf16, tag="hT")
            # v natural + ones column: [tok%128, tok//128, head, 65]
            vA = resid.tile([128, NTOK, HPC, HD + 1], bf16, tag="vA")
            nc.vector.memset(vA[:, :, :, HD:HD + 1], 1.0)

            xTr = x_in.ap().rearrange("(kt p) n -> p kt n", p=128)

            # ---- phase 1 (itemized): QKV projection + RoPE for one
            # 512-token chunk; returns a list of emission closures so the
            # scheduler can weave them between score chunks. ----
            def phase1_items(tci):
                tsl = ts(tci, TC)
                ssl = ds((tci * TC) % S, TC)
                xt = xp.tile([128, KT, TC], bf16, tag="xt")
                items = [lambda: nc.sync.dma_start(out=xt, in_=xTr[:, :, tsl])]

                def qk(w_sb, b_sb, dest):
                    # pa is reused for the rotation product pb (the rope
                    # matmul overwrites it after the bias-add drains it)
                    sub = []
                    pa = pjps.tile([128, TC], fp32, tag="pj")
                    for kt in range(KT):
                        sub.append(lambda kt=kt, pa=pa: nc.tensor.matmul(
                            pa, lhsT=w_sb[:, kt, :], rhs=xt[:, kt, :],
                            start=(kt == 0), stop=(kt == KT - 1)))
                    a_sb = wk3.tile([128, TC], bf16, tag="a_sb")

                    def fin1(pa=pa, a_sb=a_sb):
                        nc.vector.tensor_scalar_add(a_sb, pa, b_sb)
                        nc.tensor.matmul(pa, lhsT=rt_sb, rhs=a_sb,
                                         start=True, stop=True)
                    sub.append(fin1)

                    def fin2(a_sb=a_sb, pa=pa, dest=dest):
                        t1 = wk3.tile([128, TC], bf16, tag="t1")
                        nc.vector.tensor_mul(t1, a_sb, cos_sb[:, ssl])
                        t2 = wk3.tile([128, TC], bf16, tag="t2")
                        nc.vector.tensor_mul(t2, pa, sin_sb[:, ssl])
                        nc.vector.tensor_add(dest[:, tsl], t1, t2)
                    sub.append(fin2)
                    return sub

                items += qk(wq_sb, bq_sb, qT)
                items += qk(wk_sb, bk_sb, kT)

                # v: natural layout; one full-bank psum tile per token tile
                # (start=True clears the whole bank, so regions of one bank
                # cannot host independent accumulation chains)
                for sub in range(TC // 128):
                    pv = vps.tile([128, TC], fp32, tag="v",
                                  name=f"pv_{tci}_{sub}")

                    def vstep(sub=sub, pv=pv):
                        for kt in range(KT):
                            nc.tensor.matmul(
                                pv[:, 0:PC],
                                lhsT=xt[:, kt, ds(sub * 128, 128)],
                                rhs=wv_sb[:, kt, :],
                                start=(kt == 0), stop=(kt == KT - 1))
                    items.append(vstep)

                    def vdrain(pv=pv, tt=tci * (TC // 128) + sub):
                        nc.vector.tensor_add(
                            vA[:, tt, :, 0:HD],
                            pv[:, 0:PC].rearrange("p (h d) -> p h d", h=HPC),
                            bvb_sb.rearrange("p (h d) -> p h d", h=HPC))
                    items.append(vdrain)
                return items

            # ---- phase 2 slot: one (kt, q-chunk) for both heads. The two
            # K=64 matmuls land on disjoint PE row-halves (auto row
            # tiling) so they execute concurrently; one dual-head exp
            # drains both PSUM banks in a single ACT instruction. ----
            def p2_slot(b, kt, lo, hi, ptc):
                base = b * S
                w = hi - lo
                off = OFFS[kt] + lo - 128 * kt
                sc = scps.tile([128, HPC, 512], fp32, tag="sc")
                for h in range(HPC):
                    hsl = ds(h * HD, HD)
                    nc.tensor.matmul(
                        sc[:, h, 0:w],
                        lhsT=kT[hsl, ds(base + kt * 128, 128)],
                        rhs=qT[hsl, ds(base + lo, w)],
                        start=True, stop=True)
                nc.scalar.activation(
                    ptc[:, :, ds(off, w)], sc[:, :, 0:w],
                    mybir.ActivationFunctionType.Exp, scale=SCALE)
                if lo == 128 * kt:
                    # diagonal block: mask k_local > q_local on GpSimd
                    for h in range(HPC):
                        dsl = ds(OFFS[kt], 128)
                        nc.gpsimd.tensor_mul(ptc[:, h, dsl], ptc[:, h, dsl],
                                             mask_sb)

            # ---- phase 3: one PV q-window for one (b, h) ----
            def phase3(b, h, w, ptc):
                pt = ptc[:, h, :]
                kmax = (w * WW + WW - 1) // 128   # last kt contributing
                pvt = pvps.tile([128, WW], fp32, tag="pv")
                for kt in range(kmax + 1):
                    qlo = max(128 * kt, w * WW)
                    width = w * WW + WW - qlo
                    nc.tensor.matmul(
                        pvt[0:HD + 1, ds(qlo - w * WW, width)],
                        lhsT=vA[:, b * NQT + kt, h, :],
                        rhs=pt[:, ds(OFFS[kt] + qlo - 128 * kt, width)],
                        start=(kt == 0), stop=(kt == kmax))
                # drain: copy rows 0-64 (incl. sums row) to SBUF, wide
                # approx reciprocal, then broadcast the reciprocal row to 64
                # partitions via a DRAM bounce (write row, read stride-0).
                stg = pvw.tile([128, WW], fp32, tag="stg")
                nc.vector.tensor_copy(stg[0:HD + 1, :], pvt[0:HD + 1, :])
                rcp = pvw.tile([128, WW], fp32, tag="rcp")
                nc.vector.reciprocal_approx_fast(out=rcp, in_=stg)
                ri = (b * HPC + h) * NW + w
                nc.gpsimd.dma_start(out=rs_d[ri, :], in_=rcp[HD:HD + 1, :])
                rbc = pvw.tile([HD, WW], fp32, tag="rbc")
                nc.gpsimd.dma_start(
                    out=rbc,
                    in_=bass.AP(tensor=rs_d, offset=ri * WW,
                                ap=[[0, HD], [1, WW]]))
                span = ds(b * S + w * WW, WW)
                if h == 0:
                    nc.vector.tensor_mul(hT[0:HD, span], stg[0:HD, :], rbc)
                else:
                    hst = pvw.tile([HD, WW], bf16, tag="hst")
                    nc.vector.tensor_mul(hst, stg[0:HD, :], rbc)
                    nc.gpsimd.dma_start(out=hT[HD:128, span], in_=hst)

            # ---- phase 4 (itemized): output projection chunks for one
            # batch, cc-outer; woven into the next phase as PE filler ----
            def phase4_items(b):
                base = b * S
                items = []
                n = 0
                for cc in range(S // 512):
                    for ft in range(D // 128):
                        def p4(ft=ft, cc=cc, n=n):
                            po = pjps.tile([128, 512], fp32, tag="pj",
                                           name=f"po_{b}_{cc}_{ft}")
                            nc.tensor.matmul(
                                po, lhsT=wo_sb[:, ts(ft, 128)],
                                rhs=hT[:, ds(base + cc * 512, 512)],
                                start=True, stop=True)
                            ostage = ow.tile([128, 512], bf16, tag="ostage",
                                             name=f"os_{b}_{cc}_{ft}")
                            if n % 2 == 0:
                                nc.vector.tensor_copy(ostage, po)
                            else:
                                nc.scalar.copy(ostage, po)
                            nc.sync.dma_start(
                                out=out_d[ts(ft, 128),
                                          ds(base + cc * 512, 512)],
                                in_=ostage)
                        items.append(p4)
                        n += 1
                return items

            # ---- master schedule ----
            # prologue: batch 0 projection
            for it in phase1_items(0):
                it()
            for it in phase1_items(1):
                it()
            for it in phase1_items(2):
                it()
            for it in phase1_items(3):
                it()

            from collections import deque

            for b in range(B):
                # next batch's projection items, woven between score slots
                # so the PE always has queued work while ACT drains exps
                fillers = deque()
                if b + 1 < B:
                    for cc in range(CPB):
                        fillers.extend(phase1_items((b + 1) * CPB + cc))
                ptc = ptri_pool.tile([128, HPC, PTRI_W], bf16, tag="pt",
                                     name=f"pt_b{b}")
                slots_left = sum(len(_row_chunks(kt)) for kt in range(NQT))
                for g in range(NW):           # 4 row-groups of 4 kt rows
                    for kt in range(4 * g, 4 * g + 4):
                        for (lo, hi) in _row_chunks(kt):
                            p2_slot(b, kt, lo, hi, ptc)
                            if fillers and slots_left > 0:
                                want = -(-len(fillers) // slots_left)
                                for _ in range(want):
                                    if fillers:
                                        fillers.popleft()()
                            slots_left -= 1
                    # this row-group complete: its PV window is ready
                    for h in range(HPC):
                        phase3(b, h, g, ptc)
                while fillers:
                    fillers.popleft()()
                if dbg and b == 0:
                    nc.sync.dma_start(out=dbg_pt[:, :],
                                      in_=ptc.rearrange("p h w -> p (h w)"))
                for it in phase4_items(b):
                    it()
            if dbg:
                nc.sync.dma_start(out=dbg_qT[:, :], in_=qT)
                nc.sync.dma_start(out=dbg_kT[:, :], in_=kT)
                nc.sync.dma_start(out=dbg_vA[:, :],
                                  in_=vA.rearrange("p a h d -> p (a h d)"))
                nc.sync.dma_start(out=dbg_hT[:, :], in_=hT)

    nc.compile()
    return nc


_NC_CACHE = None


def _get_nc():
    global _NC_CACHE
    if _NC_CACHE is None:
        _NC_CACHE = _build_nc()
    return _NC_CACHE


def kernel(x, positions, Wqkv, bqkv, Wo, bo):
    x = np.asarray(x)
    positions = np.asarray(positions)
    Wqkv = np.asarray(Wqkv)
    bqkv = np.asarray(bqkv)
    Wo = np.asarray(Wo)
    bo = np.asarray(bo)

    nc = _get_nc()

    xT = np.ascontiguousarray(x.reshape(T, D).T).astype(BF16)
    pos = np.ascontiguousarray(positions[0]).astype(np.int32)

    in_maps = []
    for c in range(NCORES):
        r0 = c * PC
        wq = np.ascontiguousarray(Wqkv[r0:r0 + PC, :].T).astype(BF16)
        wk = np.ascontiguousarray(Wqkv[D + r0:D + r0 + PC, :].T).astype(BF16)
        wv = np.ascontiguousarray(Wqkv[2 * D + r0:2 * D + r0 + PC, :].T).astype(BF16)
        wo = np.ascontiguousarray(Wo[:, r0:r0 + PC].T).astype(BF16)
        in_maps.append({
            "x": xT, "pos": pos,
            "wq": wq, "wk": wk, "wv": wv, "wo": wo,
            "bq": bqkv[r0:r0 + PC].astype(np.float32),
            "bk": bqkv[D + r0:D + r0 + PC].astype(np.float32),
            "bv": bqkv[2 * D + r0:2 * D + r0 + PC].astype(np.float32),
        })

    res = run_bass_kernel_spmd(nc, in_maps, core_ids=list(range(NCORES)))
    acc = res.results[0]["out"].astype(np.float32)
    for c in range(1, NCORES):
        acc += res.results[c]["out"].astype(np.float32)
    out = acc + bo[:, None].astype(np.float32)
    return np.ascontiguousarray(out.T).reshape(B, S, D)


# revision 43
# speedup vs baseline: 1.0884x; 1.0221x over previous
"""Multi-head self-attention (RoPE, causal) Trainium2 Bass kernel.

Problem: B=4, S=2048, D=1024, H=16 heads, hd=64, fused QKV + RoPE +
causal softmax attention + output projection (torch-Linear convention).

Sharding: Megatron-style tensor parallel over heads. Each of the 8
NeuronCores owns 2 heads: it projects the full token stream through its
128-row slices of Wq/Wk/Wv, applies RoPE, runs causal attention for its
2 heads x 4 batches, and computes a partial output projection
h_core @ Wo[:, core_slice].T  (transposed layout). The host sums the 8
partial outputs and adds the output bias.

Per-core pipeline (restructured for PE density / HAM warmth):
  1. QKV projection per 512-token chunk: q,k produced in [d, tok] layout
     (weights stationary, wide moving streams), RoPE via a pair-rotation
     matmul + DVE combines; v in natural [tok, d] layout (x-tiles
     stationary) with a ones column appended for softmax sums.
  2. Scores transposed sT[k,q] = kT.T@qT per (batch, head): the two
     heads' K=64 matmuls are emitted pairwise so they run concurrently
     in the two row-halves of the PE array (tile_position row tiling).
     exp on ACT (scale fused); causal diagonal blocks masked on GpSimd.
  3. P@V V-stationary: out[65, q] = sum_kt vA[kt].T @ P_T[kt-row], with
     per-512-q-window PSUM accumulators; row 64 carries the softmax
     denominators. Drain: copy rows 0-63 to SBUF, reciprocal of row 64,
     GpSimd partition_broadcast of the reciprocal row, DVE multiply
     producing normalized hT[d, tok] (head 1 shifted to partitions
     64-127 by a small SBUF-to-SBUF DMA).
  4. Output projection, transposed: partial.T[f, tok] = WoT.T @ hT,
     PSUM -> SBUF casts alternating DVE/ACT, DMA to DRAM.
Emission interleaves phases so the PE queue never drains (HAM 2.4GHz).
"""

import os
import sys

for _p in ("/opt/trn_rl_repo",):
    if os.path.isdir(_p) and _p not in sys.path:
        sys.path.append(_p)

import math

import ml_dtypes
import numpy as np

import concourse.bass as bass
import concourse.mybir as mybir
import concourse.tile as tile
from concourse import bacc
from concourse.bass import ts, ds
from concourse.bass_utils import run_bass_kernel_spmd

BF16 = ml_dtypes.bfloat16

B = 4
S = 2048
D = 1024
H = 16
HD = 64
NCORES = 8
HPC = H // NCORES          # heads per core = 2
PC = HPC * HD              # partition rows per core's heads = 128
T = B * S                  # 8192 tokens
KT = D // 128              # f_in k-tiles = 8
NTOK = T // 128            # 64 token tiles of 128
SCALE = 1.0 / math.sqrt(HD)
ROPE_THETA = 10000.0

TWO_PI = 2.0 * math.pi
INV_2PI = 1.0 / TWO_PI
MAGIC = 12582912.0         # 1.5 * 2**23, float32 round-to-nearest trick
HALF_PI = math.pi / 2.0

NQT = S // 128             # 16 k/q tiles per sequence
WW = 512                   # PV q-window width
NW = S // WW               # 4 windows per (batch, head)
# triangular packing offsets for P_T: row kt covers q in [kt*128, S)
OFFS = [0] * NQT
for _kt in range(1, NQT):
    OFFS[_kt] = OFFS[_kt - 1] + (S - (_kt - 1) * 128)
PTRI_W = OFFS[-1] + (S - (NQT - 1) * 128)   # 17408

TC = 512                   # projection token chunk
NTC = T // TC              # 16 chunks
CPB = S // TC              # chunks per batch = 4


def _row_chunks(kt):
    """512-grid-aligned q chunks covering [128*kt, S)."""
    out = []
    lo = 128 * kt
    while lo < S:
        hi = min(S, (lo // 512 + 1) * 512)
        out.append((lo, hi))
        lo = hi
    return out


def _build_nc(dbg=False):
    nc = bacc.Bacc("TRN2", target_bir_lowering=False, debug=False,
                   num_devices=NCORES)
    dt = mybir.dt

    # ---- I/O ----
    x_in = nc.dram_tensor("x", [D, T], dt.bfloat16, kind="ExternalInput")
    pos_in = nc.dram_tensor("pos", [S], dt.int32, kind="ExternalInput")
    wq_in = nc.dram_tensor("wq", [D, PC], dt.bfloat16, kind="ExternalInput")
    wk_in = nc.dram_tensor("wk", [D, PC], dt.bfloat16, kind="ExternalInput")
    wv_in = nc.dram_tensor("wv", [D, PC], dt.bfloat16, kind="ExternalInput")
    wo_in = nc.dram_tensor("wo", [PC, D], dt.bfloat16, kind="ExternalInput")
    bq_in = nc.dram_tensor("bq", [PC], dt.float32, kind="ExternalInput")
    bk_in = nc.dram_tensor("bk", [PC], dt.float32, kind="ExternalInput")
    bv_in = nc.dram_tensor("bv", [PC], dt.float32, kind="ExternalInput")
    out_d = nc.dram_tensor("out", [D, T], dt.bfloat16, kind="ExternalOutput")
    # DRAM scratch for the per-window softmax-reciprocal rows (bounce buffer
    # for the partition broadcast; SBUF stride-0 DMA is not supported)
    rs_d = nc.dram_tensor("rscratch", [B * HPC * NW, WW], dt.float32,
                          kind="Internal")
    if dbg:
        dbg_qT = nc.dram_tensor("dbg_qT", [128, T], dt.bfloat16,
                                kind="ExternalOutput")
        dbg_kT = nc.dram_tensor("dbg_kT", [128, T], dt.bfloat16,
                                kind="ExternalOutput")
        dbg_vA = nc.dram_tensor("dbg_vA", [128, NTOK * HPC * (HD + 1)],
                                dt.bfloat16, kind="ExternalOutput")
        dbg_pt = nc.dram_tensor("dbg_pt", [128, HPC * PTRI_W], dt.bfloat16,
                                kind="ExternalOutput")
        dbg_hT = nc.dram_tensor("dbg_hT", [128, T], dt.bfloat16,
                                kind="ExternalOutput")

    # ---- inline constants ----
    # RT = R.T where (R @ q)[2i] = -q[2i+1], (R @ q)[2i+1] = q[2i],
    # block-diagonal over the 2 stacked heads.
    r = np.zeros((PC, PC), dtype=np.float32)
    for h in range(HPC):
        for i in range(HD // 2):
            r[h * HD + 2 * i, h * HD + 2 * i + 1] = -1.0
            r[h * HD + 2 * i + 1, h * HD + 2 * i] = 1.0
    rt_np = np.ascontiguousarray(r.T).astype(BF16)
    # causal mask for diagonal scoresT blocks: keep k_local <= q_local
    mask_np = np.tril(np.ones((128, 128), dtype=np.float32)).T.astype(BF16)
    # inv_freq per partition: p -> head-local pair (p % 64) // 2
    invf_np = np.zeros((PC, 1), dtype=np.float32)
    for p in range(PC):
        i = (p % HD) // 2
        invf_np[p, 0] = 1.0 / (ROPE_THETA ** (2.0 * i / HD))

    rt_d = nc.inline_tensor(rt_np, "rt_c")
    mask_d = nc.inline_tensor(mask_np, "mask_c")
    invf_d = nc.inline_tensor(invf_np, "invf_c")

    fp32 = dt.float32
    bf16 = dt.bfloat16

    with tile.TileContext(nc) as tc:
        with (
            tc.tile_pool(name="consts", bufs=1) as consts,
            tc.tile_pool(name="resid", bufs=1) as resid,
            tc.tile_pool(name="xp", bufs=2) as xp,
            tc.tile_pool(name="wk3", bufs=3) as wk3,
            tc.tile_pool(name="pvw", bufs=2) as pvw,
            tc.tile_pool(name="ow", bufs=3) as ow,
            tc.tile_pool(name="csw", bufs=1) as csw,
            tc.tile_pool(name="ptri", bufs=1) as ptri_pool,
            tc.tile_pool(name="scps", bufs=2, space="PSUM") as scps,
            tc.tile_pool(name="pvps", bufs=2, space="PSUM") as pvps,
            tc.tile_pool(name="pjps", bufs=2, space="PSUM") as pjps,
        ):
            # ---- load constants / weights to SBUF ----
            wq_sb = consts.tile([128, KT, PC], bf16, tag="wq")
            wk_sb = consts.tile([128, KT, PC], bf16, tag="wk")
            wv_sb = consts.tile([128, KT, PC], bf16, tag="wv")
            for t_sb, t_d in ((wq_sb, wq_in), (wk_sb, wk_in), (wv_sb, wv_in)):
                nc.sync.dma_start(
                    out=t_sb, in_=t_d.ap().rearrange("(kt p) m -> p kt m", p=128))
            wo_sb = consts.tile([128, D], bf16, tag="wo")
            nc.sync.dma_start(out=wo_sb, in_=wo_in[:, :])
            rt_sb = consts.tile([128, 128], bf16, tag="rt")
            nc.sync.dma_start(out=rt_sb, in_=rt_d[:, :])
            mask_sb = consts.tile([128, 128], bf16, tag="mask")
            nc.sync.dma_start(out=mask_sb, in_=mask_d[:, :])
            invf_sb = consts.tile([128, 1], fp32, tag="invf")
            nc.sync.dma_start(out=invf_sb, in_=invf_d[:, :])
            bq_sb = consts.tile([128, 1], fp32, tag="bq")
            nc.sync.dma_start(out=bq_sb, in_=bq_in.ap().rearrange("(p o) -> p o", o=1))
            bk_sb = consts.tile([128, 1], fp32, tag="bk")
            nc.sync.dma_start(out=bk_sb, in_=bk_in.ap().rearrange("(p o) -> p o", o=1))
            # v bias broadcast over token partitions: [128, PC] f32
            bvb_sb = consts.tile([128, PC], fp32, tag="bvb")
            nc.sync.dma_start(
                out=bvb_sb,
                in_=bass.AP(tensor=bv_in, offset=0, ap=[[0, 128], [1, PC]]))
            halfpi_sb = consts.tile([128, 1], fp32, tag="halfpi")
            nc.vector.memset(halfpi_sb, HALF_PI)

            # ---- RoPE cos/sin tables [128, S] bf16, from positions ----
            cos_sb = consts.tile([128, S], bf16, tag="cosT")
            sin_sb = consts.tile([128, S], bf16, tag="sinT")
            CS_CH = 512
            for ci in range(S // CS_CH):
                sl = ts(ci, CS_CH)
                posi = csw.tile([128, CS_CH], dt.int32, tag="posi")
                nc.sync.dma_start(
                    out=posi,
                    in_=bass.AP(tensor=pos_in, offset=ci * CS_CH,
                                ap=[[0, 128], [1, CS_CH]]))
                posf = csw.tile([128, CS_CH], fp32, tag="posf")
                nc.vector.tensor_copy(posf, posi)
                ang = csw.tile([128, CS_CH], fp32, tag="ang")
                nc.vector.tensor_scalar_mul(ang, posf, invf_sb)
                # sin: reduce ang to [-pi, pi]
                rnd = csw.tile([128, CS_CH], fp32, tag="rnd")
                red = csw.tile([128, CS_CH], fp32, tag="red")
                nc.vector.tensor_scalar(rnd, ang, INV_2PI, MAGIC,
                                        mybir.AluOpType.mult,
                                        mybir.AluOpType.add)
                nc.vector.tensor_scalar(rnd, rnd, MAGIC, None,
                                        mybir.AluOpType.subtract)
                nc.vector.scalar_tensor_tensor(
                    red, rnd, -TWO_PI, ang,
                    op0=mybir.AluOpType.mult, op1=mybir.AluOpType.add)
                nc.scalar.activation(sin_sb[:, sl], red,
                                     mybir.ActivationFunctionType.Sin)
                # cos(x) = sin(y + pi/2), y = x - 2pi*round((x+pi/2)/2pi)
                nc.vector.tensor_scalar(rnd, ang, INV_2PI, MAGIC + 0.25,
                                        mybir.AluOpType.mult,
                                        mybir.AluOpType.add)
                nc.vector.tensor_scalar(rnd, rnd, MAGIC, None,
                                        mybir.AluOpType.subtract)
                nc.vector.scalar_tensor_tensor(
                    red, rnd, -TWO_PI, ang,
                    op0=mybir.AluOpType.mult, op1=mybir.AluOpType.add)
                nc.scalar.activation(cos_sb[:, sl], red,
                                     mybir.ActivationFunctionType.Sin,
                                     bias=halfpi_sb[:, :])

            # ---- residents ----
            qT = resid.tile([128, T], bf16, tag="qT")     # [d(2h), tok]
            kT = resid.tile([128, T], bf16, tag="kT")
            hT = resid.tile([128, T], bf16, tag="hT")
            # v natural + ones column: [tok%128, tok//128, head, 65]
            vA = resid.tile([128, NTOK, HPC, HD + 1], bf16, tag="vA")
            nc.vector.memset(vA[:, :, :, HD:HD + 1], 1.0)

            xTr = x_in.ap().rearrange("(kt p) n -> p kt n", p=128)

            # ---- phase 1 (itemized): QKV projection + RoPE for one
            # 512-token chunk; returns a list of emission closures so the
            # scheduler can weave them between score chunks. ----
            def phase1_items(tci):
                tsl = ts(tci, TC)
                ssl = ds((tci * TC) % S, TC)
                xt = xp.tile([128, KT, TC], bf16, tag="xt")
                items = [lambda: nc.sync.dma_start(out=xt, in_=xTr[:, :, tsl])]

                def qk(w_sb, b_sb, dest):
                    # pa is reused for the rotation product pb (the rope
                    # matmul overwrites it after the bias-add drains it)
                    sub = []
                    pa = pjps.tile([128, TC], fp32, tag="pj")
                    for kt in range(KT):
                        sub.append(lambda kt=kt, pa=pa: nc.tensor.matmul(
                            pa, lhsT=w_sb[:, kt, :], rhs=xt[:, kt, :],
                            start=(kt == 0), stop=(kt == KT - 1)))
                    a_sb = wk3.tile([128, TC], bf16, tag="a_sb")

                    def fin1(pa=pa, a_sb=a_sb):
                        nc.vector.tensor_scalar_add(a_sb, pa, b_sb)
                        nc.tensor.matmul(pa, lhsT=rt_sb, rhs=a_sb,
                                         start=True, stop=True)
                    sub.append(fin1)

                    def fin2(a_sb=a_sb, pa=pa, dest=dest):
                        t1 = wk3.tile([128, TC], bf16, tag="t1")
                        nc.vector.tensor_mul(t1, a_sb, cos_sb[:, ssl])
                        t2 = wk3.tile([128, TC], bf16, tag="t2")
                        nc.vector.tensor_mul(t2, pa, sin_sb[:, ssl])
                        nc.vector.tensor_add(dest[:, tsl], t1, t2)
                    sub.append(fin2)
                    return sub

                items += qk(wq_sb, bq_sb, qT)
                items += qk(wk_sb, bk_sb, kT)

                # v: natural layout; one full-bank psum tile per token tile
                # (start=True clears the whole bank, so regions of one bank
                # cannot host independent accumulation chains)
                for sub in range(TC // 128):
                    pv = pjps.tile([128, TC], fp32, tag="pj",
                                   name=f"pv_{tci}_{sub}")

                    def vstep(sub=sub, pv=pv):
                        for kt in range(KT):
                            nc.tensor.matmul(
                                pv[:, 0:PC],
                                lhsT=xt[:, kt, ds(sub * 128, 128)],
                                rhs=wv_sb[:, kt, :],
                                start=(kt == 0), stop=(kt == KT - 1))
                    items.append(vstep)

                    def vdrain(pv=pv, tt=tci * (TC // 128) + sub):
                        nc.vector.tensor_add(
                            vA[:, tt, :, 0:HD],
                            pv[:, 0:PC].rearrange("p (h d) -> p h d", h=HPC),
                            bvb_sb.rearrange("p (h d) -> p h d", h=HPC))
                    items.append(vdrain)
                return items

            # ---- phase 2 slot: one (kt, q-chunk) for both heads. The two
            # K=64 matmuls land on disjoint PE row-halves (auto row
            # tiling) so they execute concurrently; one dual-head exp
            # drains both PSUM banks in a single ACT instruction. ----
            def p2_slot(b, kt, lo, hi, ptc):
                base = b * S
                w = hi - lo
                off = OFFS[kt] + lo - 128 * kt
                sc = scps.tile([128, HPC, 512], fp32, tag="sc")
                for h in range(HPC):
                    hsl = ds(h * HD, HD)
                    nc.tensor.matmul(
                        sc[:, h, 0:w],
                        lhsT=kT[hsl, ds(base + kt * 128, 128)],
                        rhs=qT[hsl, ds(base + lo, w)],
                        start=True, stop=True)
                nc.scalar.activation(
                    ptc[:, :, ds(off, w)], sc[:, :, 0:w],
                    mybir.ActivationFunctionType.Exp, scale=SCALE)
                if lo == 128 * kt:
                    # diagonal block: mask k_local > q_local on GpSimd
                    for h in range(HPC):
                        dsl = ds(OFFS[kt], 128)
                        nc.gpsimd.tensor_mul(ptc[:, h, dsl], ptc[:, h, dsl],
                                             mask_sb)

            # ---- phase 3: one PV q-window for one (b, h) ----
            def phase3(b, h, w, ptc):
                pt = ptc[:, h, :]
                kmax = (w * WW + WW - 1) // 128   # last kt contributing
                pvt = pvps.tile([128, WW], fp32, tag="pv")
                for kt in range(kmax + 1):
                    qlo = max(128 * kt, w * WW)
                    width = w * WW + WW - qlo
                    nc.tensor.matmul(
                        pvt[0:HD + 1, ds(qlo - w * WW, width)],
                        lhsT=vA[:, b * NQT + kt, h, :],
                        rhs=pt[:, ds(OFFS[kt] + qlo - 128 * kt, width)],
                        start=(kt == 0), stop=(kt == kmax))
                # drain: copy rows 0-64 (incl. sums row) to SBUF, wide
                # approx reciprocal, then broadcast the reciprocal row to 64
                # partitions via a DRAM bounce (write row, read stride-0).
                stg = pvw.tile([128, WW], fp32, tag="stg")
                nc.vector.tensor_copy(stg[0:HD + 1, :], pvt[0:HD + 1, :])
                rcp = pvw.tile([128, WW], fp32, tag="rcp")
                nc.vector.reciprocal_approx_fast(out=rcp, in_=stg)
                ri = (b * HPC + h) * NW + w
                nc.gpsimd.dma_start(out=rs_d[ri, :], in_=rcp[HD:HD + 1, :])
                rbc = pvw.tile([HD, WW], fp32, tag="rbc")
                nc.gpsimd.dma_start(
                    out=rbc,
                    in_=bass.AP(tensor=rs_d, offset=ri * WW,
                                ap=[[0, HD], [1, WW]]))
                span = ds(b * S + w * WW, WW)
                if h == 0:
                    nc.vector.tensor_mul(hT[0:HD, span], stg[0:HD, :], rbc)
                else:
                    hst = pvw.tile([HD, WW], bf16, tag="hst")
                    nc.vector.tensor_mul(hst, stg[0:HD, :], rbc)
                    nc.gpsimd.dma_start(out=hT[HD:128, span], in_=hst)

            # ---- phase 4 (itemized): output projection chunks for one
            # batch, cc-outer; woven into the next phase as PE filler ----
            def phase4_items(b):
                base = b * S
                items = []
                n = 0
                for cc in range(S // 512):
                    for ft in range(D // 128):
                        def p4(ft=ft, cc=cc, n=n):
                            po = pjps.tile([128, 512], fp32, tag="pj",
                                           name=f"po_{b}_{cc}_{ft}")
                            nc.tensor.matmul(
                                po, lhsT=wo_sb[:, ts(ft, 128)],
                                rhs=hT[:, ds(base + cc * 512, 512)],
                                start=True, stop=True)
                            ostage = ow.tile([128, 512], bf16, tag="ostage",
                                             name=f"os_{b}_{cc}_{ft}")
                            if n % 2 == 0:
                                nc.vector.tensor_copy(ostage, po)
                            else:
                                nc.scalar.copy(ostage, po)
                            nc.sync.dma_start(
                                out=out_d[ts(ft, 128),
                                          ds(base + cc * 512, 512)],
                                in_=ostage)
                        items.append(p4)
                        n += 1
                return items

            # ---- master schedule ----
            # prologue: batch 0 projection
            for it in phase1_items(0):
                it()
            for it in phase1_items(1):
                it()
            for it in phase1_items(2):
                it()
            for it in phase1_items(3):
                it()

            from collections import deque

            for b in range(B):
                # next batch's projection items, woven between score slots
                # so the PE always has queued work while ACT drains exps
                fillers = deque()
                if b + 1 < B:
                    for cc in range(CPB):
                        fillers.extend(phase1_items((b + 1) * CPB + cc))
                ptc = ptri_pool.tile([128, HPC, PTRI_W], bf16, tag="pt",
                                     name=f"pt_b{b}")
                slots_left = sum(len(_row_chunks(kt)) for kt in range(NQT))
                for g in range(NW):           # 4 row-groups of 4 kt rows
                    for kt in range(4 * g, 4 * g + 4):
                        for (lo, hi) in _row_chunks(kt):
                            p2_slot(b, kt, lo, hi, ptc)
                            if fillers and slots_left > 0:
                                want = -(-len(fillers) // slots_left)
                                for _ in range(want):
                                    if fillers:
                                        fillers.popleft()()
                            slots_left -= 1
                    # this row-group complete: its PV window is ready
                    for h in range(HPC):
                        phase3(b, h, g, ptc)
                while fillers:
                    fillers.popleft()()
                if dbg and b == 0:
                    nc.sync.dma_start(out=dbg_pt[:, :],
                                      in_=ptc.rearrange("p h w -> p (h w)"))
                for it in phase4_items(b):
                    it()
            if dbg:
                nc.sync.dma_start(out=dbg_qT[:, :], in_=qT)
                nc.sync.dma_start(out=dbg_kT[:, :], in_=kT)
                nc.sync.dma_start(out=dbg_vA[:, :],
                                  in_=vA.rearrange("p a h d -> p (a h d)"))
                nc.sync.dma_start(out=dbg_hT[:, :], in_=hT)

    nc.compile()
    return nc


_NC_CACHE = None


def _get_nc():
    global _NC_CACHE
    if _NC_CACHE is None:
        _NC_CACHE = _build_nc()
    return _NC_CACHE


def kernel(x, positions, Wqkv, bqkv, Wo, bo):
    x = np.asarray(x)
    positions = np.asarray(positions)
    Wqkv = np.asarray(Wqkv)
    bqkv = np.asarray(bqkv)
    Wo = np.asarray(Wo)
    bo = np.asarray(bo)

    nc = _get_nc()

    xT = np.ascontiguousarray(x.reshape(T, D).T).astype(BF16)
    pos = np.ascontiguousarray(positions[0]).astype(np.int32)

    in_maps = []
    for c in range(NCORES):
        r0 = c * PC
        wq = np.ascontiguousarray(Wqkv[r0:r0 + PC, :].T).astype(BF16)
        wk = np.ascontiguousarray(Wqkv[D + r0:D + r0 + PC, :].T).astype(BF16)
        wv = np.ascontiguousarray(Wqkv[2 * D + r0:2 * D + r0 + PC, :].T).astype(BF16)
        wo = np.ascontiguousarray(Wo[:, r0:r0 + PC].T).astype(BF16)
        in_maps.append({
            "x": xT, "pos": pos,
            "wq": wq, "wk": wk, "wv": wv, "wo": wo,
            "bq": bqkv[r0:r0 + PC].astype(np.float32),
            "bk": bqkv[D + r0:D + r0 + PC].astype(np.float32),
            "bv": bqkv[2 * D + r0:2 * D + r0 + PC].astype(np.float32),
        })

    res = run_bass_kernel_spmd(nc, in_maps, core_ids=list(range(NCORES)))
    acc = res.results[0]["out"].astype(np.float32)
    for c in range(1, NCORES):
        acc += res.results[c]["out"].astype(np.float32)
    out = acc + bo[:, None].astype(np.float32)
    return np.ascontiguousarray(out.T).reshape(B, S, D)
